# revision 52
# baseline (speedup 1.0000x reference)
"""CLUB loss kernel for Trainium2, 8-core data-parallel SPMD.

Math: with flat_x (N,D) [from x (B,D,H,W) -> (B*H*W, D)], v = exp(-p_logvar),
  loss = mean_i[ -0.5*sum_d ((x-mu)^2 - (m2 - 2*mu*m1 + mu^2)) * v ]
       = (-0.5/N) * [ A - 2B - dot(m2, V) + 2*dot(m1, W) ]
where
  A  = sum_{i,d} x^2 v          B  = sum_{i,d} x mu v
  V_d = sum_i v                 W_d = sum_i mu v
  m1 = S1/N, m2 = S2/N,  S1_d = sum_i x,  S2_d = sum_i x^2
All terms are per-core-local partial sums; the tiny (~KB) cross-core
reduction and final dot products happen on host in float64. No collectives.

Layout: d-major (partition = d) so every reduction above is a free-axis sum
riding on `accum_out` of ops we need anyway; PE does only 128x128 block
transposes of mu/logvar (identity matmuls into PSUM) and no reduction
matmuls. Engine split: ACT = exp (fused with lvT PSUM evacuation + V), x^2
(+S2), copy (+S1) — the x-only passes run early; DVE = the three product
passes w/a/b with their W/A/B reductions fused, kept per-512-wide half so
the post-last-DMA tail stays short.

Streaming: 512 KiB mu/lv slabs + 512 KiB x quarters issued interleaved so
complete (b-block, i-half, d-chunk) work becomes ready uniformly across the
~36 us DMA window and the engines pipeline directly behind the stream.
"""

import sys

import numpy as np

for _p in ("/opt/trn_rl_repo",):
    if _p not in sys.path:
        sys.path.append(_p)

B, D, H, W = 16, 512, 32, 32
HW = H * W
N = B * HW
NCORES = 8
BLKB = B // NCORES          # b-blocks per core (2)
ROWS = N // NCORES          # rows per core (2048)
NT = ROWS // 128            # 128-row i-tiles per core (16)
NDC = D // 128              # d chunks (4)
SLAB = 2                    # i-tiles per mu/lv DMA slab (= 256 i, 512 KiB)
NU = BLKB * NDC             # full units per core (8)
HHW = HW // 2               # i-extent of a half-unit (512)

_prog_cache = {}


def build_program():
    import concourse.bacc as bacc
    import concourse.tile as tile
    from concourse import mybir

    f32 = mybir.dt.float32
    AF = mybir.ActivationFunctionType
    OP = mybir.AluOpType

    nc = bacc.Bacc(
        "TRN2",
        target_bir_lowering=False,
        debug=False,
        enable_asserts=False,
        num_devices=NCORES,
    )

    x_d = nc.dram_tensor("x_s", (BLKB, D, HW), f32, kind="ExternalInput").ap()
    mu_d = nc.dram_tensor("mu_s", (ROWS, D), f32, kind="ExternalInput").ap()
    lv_d = nc.dram_tensor("lv_s", (ROWS, D), f32, kind="ExternalInput").ap()
    id_d = nc.dram_tensor("ident", (128, 128), f32, kind="ExternalInput").ap()

    # o_misc columns (partition p -> d = 128*dc+p), unit u = b*NDC+dc,
    # half-col hc = 2*u+h:
    #   [0,2NU) V | [2NU,4NU) W | [4NU,6NU) A | [6NU,8NU) B   (per half)
    #   [8NU,9NU) S1 | [9NU,10NU) S2                          (per unit)
    o_misc = nc.dram_tensor("o_misc", (128, 10 * NU), f32, kind="ExternalOutput").ap()

    with tile.TileContext(nc) as tc:
        with (
            tc.tile_pool(name="const", bufs=1) as constp,
            tc.tile_pool(name="xnat", bufs=1) as xp,
            tc.tile_pool(name="slab", bufs=8) as slp,
            tc.tile_pool(name="vw", bufs=6) as vwp,
            tc.tile_pool(name="ppool", bufs=8) as ppool,
            tc.tile_pool(name="scr", bufs=4) as scrp,
            tc.tile_pool(name="accum", bufs=1) as accp,
            tc.tile_pool(name="psum", bufs=4, space="PSUM") as pp,
        ):
            ident = constp.tile([128, 128], f32)

            acc = [
                accp.tile([128, w_], f32, tag=f"acc{q}", name=f"acc{q}")
                for q, w_ in enumerate((2 * NU, 2 * NU, 2 * NU, 2 * NU, NU, NU))
            ]

            lv_slabs = {}
            mu_slabs = {}
            xq = {}

            def load_slab(dram, store, sidx, tag, eng=None):
                rows = dram[128 * SLAB * sidx : 128 * SLAB * (sidx + 1), :]
                t_ = slp.tile([128, SLAB * D], f32, tag=tag, name=tag)
                (eng or nc.gpsimd).dma_start(
                    t_[:], rows.rearrange("(g p) f -> p g f", p=128)
                )
                store[sidx] = t_

            def load_x_quarter(b, dc):
                t_ = xp.tile([128, HW], f32, tag=f"x_{b}_{dc}", name=f"x_{b}_{dc}")
                nc.sync.dma_start(t_[:], x_d[b, 128 * dc : 128 * (dc + 1), :])
                xq[(b, dc)] = t_

            def load_half_block(b, h, eng=None):
                # slabs covering i-tiles [8b+4h, 8b+4h+4) = 2 slabs per tensor
                s0 = (8 * b + 4 * h) // SLAB
                for s in (s0, s0 + 1):
                    load_slab(lv_d, lv_slabs, s, "lv_sl", eng)
                for s in (s0, s0 + 1):
                    load_slab(mu_d, mu_slabs, s, "mu_sl", eng)

            # interleaved issue order for uniform readiness; the very first
            # lv slab rides the Sync sequencer (earliest boot) ahead of the
            # identity and x loads so transposes can start sooner
            load_slab(lv_d, lv_slabs, 0, "lv_sl", eng=nc.sync)
            nc.sync.dma_start(ident[:], id_d[:])
            load_slab(lv_d, lv_slabs, 1, "lv_sl", eng=nc.sync)
            load_slab(mu_d, mu_slabs, 0, "mu_sl")
            load_slab(mu_d, mu_slabs, 1, "mu_sl")
            for dc in range(NDC):
                load_x_quarter(0, dc)
            load_half_block(0, 1)

            phold = {}
            for b in range(BLKB):
                for h in range(2):
                    if b > 0 and h == 0:
                        # x quarters ahead of the slabs: their x-only ACT
                        # passes must not queue behind slab-gated exps
                        for dc in range(NDC):
                            load_x_quarter(b, dc)
                        load_half_block(b, 0)
                    if b > 0 and h == 1:
                        load_half_block(b, 1)

                    for dc in range(NDC):
                        u = b * NDC + dc
                        hc = 2 * u + h
                        xs = xq[(b, dc)][:, HHW * h : HHW * (h + 1)]

                        if h == 0 and b > 0:
                            # later blocks: emit the x-only ACT passes FIRST
                            # so ACT's in-order queue can run them the moment
                            # x lands instead of stalling behind exp
                            xfull = xq[(b, dc)][:]
                            p_u = ppool.tile([128, HW], f32, tag="p", name="p_u")
                            phold[dc] = p_u
                            nc.scalar.activation(
                                p_u[:], xfull, AF.Square,
                                accum_out=acc[5][:, u : u + 1],
                            )
                            s1scr = scrp.tile(
                                [128, HW], f32, tag="s1scr", name="s1scr"
                            )
                            nc.scalar.activation(
                                s1scr[:], xfull, AF.Copy,
                                accum_out=acc[4][:, u : u + 1],
                            )

                        # asymmetric depth: lvT is freed by exp immediately,
                        # muT lives until the w-pass -> give mu the deeper run
                        lvT = pp.tile([128, HHW], f32, tag="lvT", name="lvT",
                                      bufs=3)
                        muT = pp.tile([128, HHW], f32, tag="muT", name="muT",
                                      bufs=5)
                        for blk in range(4):
                            t_i = 8 * b + 4 * h + blk
                            sl_col = D * (t_i % SLAB) + 128 * dc
                            for dst, store in ((lvT, lv_slabs), (muT, mu_slabs)):
                                nc.tensor.matmul(
                                    dst[:, 128 * blk : 128 * (blk + 1)],
                                    store[t_i // SLAB][:, sl_col : sl_col + 128],
                                    ident[:],
                                    is_transpose=True,
                                    start=(blk == 0),
                                    stop=(blk == 3),
                                )

                        v_u = vwp.tile([128, HHW], f32, tag="v", name="v_u")
                        nc.scalar.activation(
                            v_u[:], lvT[:], AF.Exp, scale=-1.0,
                            accum_out=acc[0][:, hc : hc + 1],
                        )
                        w_u = vwp.tile([128, HHW], f32, tag="w", name="w_u")
                        nc.vector.scalar_tensor_tensor(
                            out=w_u[:], in0=muT[:], scalar=1.0, in1=v_u[:],
                            op0=OP.mult, op1=OP.mult,
                            accum_out=acc[1][:, hc : hc + 1],
                        )

                        if h == 0 and b == 0:
                            # first block: x lands after lv, keep exp first
                            xfull = xq[(b, dc)][:]
                            p_u = ppool.tile([128, HW], f32, tag="p", name="p_u")
                            phold[dc] = p_u
                            nc.scalar.activation(
                                p_u[:], xfull, AF.Square,
                                accum_out=acc[5][:, u : u + 1],
                            )
                            s1scr = scrp.tile(
                                [128, HW], f32, tag="s1scr", name="s1scr"
                            )
                            nc.scalar.activation(
                                s1scr[:], xfull, AF.Copy,
                                accum_out=acc[4][:, u : u + 1],
                            )

                        ph = phold[dc][:, HHW * h : HHW * (h + 1)]
                        a_scr = scrp.tile([128, HHW], f32, tag="a", name="a_scr")
                        nc.vector.scalar_tensor_tensor(
                            out=a_scr[:], in0=ph, scalar=1.0, in1=v_u[:],
                            op0=OP.mult, op1=OP.mult,
                            accum_out=acc[2][:, hc : hc + 1],
                        )
                        b_scr = scrp.tile([128, HHW], f32, tag="b", name="b_scr")
                        nc.vector.scalar_tensor_tensor(
                            out=b_scr[:], in0=w_u[:], scalar=1.0, in1=xs,
                            op0=OP.mult, op1=OP.mult,
                            accum_out=acc[3][:, hc : hc + 1],
                        )

            off = 0
            for q in (0, 1, 2, 3, 4, 5):
                w_ = acc[q].shape[1]
                nc.sync.dma_start(o_misc[:, off : off + w_], acc[q][:])
                off += w_

    nc.compile()
    return nc


def get_program():
    if "nc" not in _prog_cache:
        _prog_cache["nc"] = build_program()
    return _prog_cache["nc"]


def make_in_maps(x, p_mu, p_logvar):
    x = np.ascontiguousarray(np.asarray(x, dtype=np.float32)).reshape(B, D, HW)
    p_mu = np.ascontiguousarray(np.asarray(p_mu, dtype=np.float32))
    p_logvar = np.ascontiguousarray(np.asarray(p_logvar, dtype=np.float32))
    in_maps = []
    for c in range(NCORES):
        in_maps.append(
            {
                "x_s": np.ascontiguousarray(x[BLKB * c : BLKB * (c + 1)]),
                "mu_s": np.ascontiguousarray(p_mu[ROWS * c : ROWS * (c + 1)]),
                "lv_s": np.ascontiguousarray(p_logvar[ROWS * c : ROWS * (c + 1)]),
                "ident": np.eye(128, dtype=np.float32),
            }
        )
    return in_maps


def finish_host(results):
    """Combine per-core partials (float64) into the scalar loss."""
    Vv = np.zeros(D)
    Ww = np.zeros(D)
    S2 = np.zeros(D)
    S1 = np.zeros(D)
    A = 0.0
    Bb = 0.0
    for r in results:
        misc = r["o_misc"].astype(np.float64)
        for u in range(NU):
            b, dc = divmod(u, NDC)
            dsl = slice(128 * dc, 128 * (dc + 1))
            for h in range(2):
                hc = 2 * u + h
                Vv[dsl] += misc[:, hc]
                Ww[dsl] += misc[:, 2 * NU + hc]
                A += float(misc[:, 4 * NU + hc].sum())
                Bb += float(misc[:, 6 * NU + hc].sum())
            S1[dsl] += misc[:, 8 * NU + u]
            S2[dsl] += misc[:, 9 * NU + u]
    m1 = S1 / N
    m2 = S2 / N
    S = A - 2.0 * Bb - float(np.dot(m2, Vv)) + 2.0 * float(np.dot(m1, Ww))
    return np.float32(-0.5 / N * S)


def run_on_device(x, p_mu, p_logvar, trace=False, **kw):
    from concourse import bass_utils

    nc = get_program()
    in_maps = make_in_maps(x, p_mu, p_logvar)
    return bass_utils.run_bass_kernel_spmd(
        nc, in_maps, list(range(NCORES)), trace=trace, **kw
    )


def kernel(x, p_mu, p_logvar):
    res = run_on_device(x, p_mu, p_logvar)
    return finish_host(res.results)
